# revision 1
# baseline (speedup 1.0000x reference)
"""Cross-attention kernel for Trainium2, sharded over 8 NeuronCores.

Problem (hardcoded shapes):
  x:       (4, 512, 2048)  queries, layout (b, dim, n)
  context: (4, 512, 2048)  keys/values source, layout (b, ctx_dim, m)
  W_q:     (512, 512), W_kv: (512, 1024), W_out: (512, 512), b_out: (512,)
  out = swapaxes(softmax((xs@Wq*scale) @ (cs@Wk)^T) @ (cs@Wv) @ Wout + b_out)

Sharding: 8 cores = 4 batches x 2 query-halves. Each core computes the
full 8-head attention for its (batch, 1024-query-slice) and produces the
exact output slice out[b][:, half] -- no cross-core reduction needed.

Per-core dataflow (fp32 storage; matmuls run in float32r = TF32-like):
  qT[inner, i]  = Wq^T @ x_slice          (PE, K=dim)
  kT[inner, j]  = Wk^T @ ctx              (PE)
  v[j, inner+ones] = ctx^T @ Wv           (PE; 65-stride layout w/ ones col)
  per head, streamed over 16 j-blocks (both 512-wide i-tiles together):
      sT[j_blk, 0:1024] = kT_h^T @ qT_h   (PE, K=64, two 512-wide matmuls)
      p = exp(sT)                         (ACT, one 1024-wide op from PSUM)
      av_it[65, 512] += v_aug^T @ p_it    (PE accumulate; row 64 = sum_j p)
  outT_h[d, i] = av[0:64] * bcast(1/av[64])  (DVE recip + DMA row-bcast + mul)
  out[dim, i]  = Wout^T @ outT + b_out    (PE + DVE bias add)
"""

import os
import sys

sys.path.insert(0, "/opt/trn_rl_repo")

import numpy as np

B, DIM, N = 4, 512, 2048
CTX_DIM, CTX_LEN = 512, 2048
H, DH, INNER = 8, 64, 512
SCALE = DH ** -0.5

NCORES = 8
NI = 1024          # query rows per core
CB = DIM // 128    # 4 partition blocks of the feature/inner dims
IT = NI // 512     # 2 i-tiles
JT = CTX_LEN // 512  # 4 j-tiles
JB = CTX_LEN // 128  # 16 j-blocks

_CACHE = {}


def _build():
    import concourse.mybir as mybir
    from concourse import bacc
    from concourse.tile import TileContext

    f32 = mybir.dt.float32
    f32r = mybir.dt.float32r
    mm_dt = f32 if os.environ.get("ATTN_MM_FP32") else f32r
    Exp = mybir.ActivationFunctionType.Exp

    nc = bacc.Bacc("TRN2", target_bir_lowering=False, debug=False)

    x_d = nc.dram_tensor("x", [DIM, NI], mm_dt, kind="ExternalInput").ap()
    ctx_d = nc.dram_tensor("ctx", [CTX_DIM, CTX_LEN], mm_dt, kind="ExternalInput").ap()
    wq_d = nc.dram_tensor("wq", [DIM, INNER], mm_dt, kind="ExternalInput").ap()
    wkv_d = nc.dram_tensor("wkv", [CTX_DIM, 2 * INNER], mm_dt, kind="ExternalInput").ap()
    wout_d = nc.dram_tensor("wout", [INNER, DIM], mm_dt, kind="ExternalInput").ap()
    bout_d = nc.dram_tensor("bout", [DIM], f32, kind="ExternalInput").ap()
    out_d = nc.dram_tensor("out", [DIM, NI], f32, kind="ExternalOutput").ap()

    with TileContext(nc) as tc:
        with (
            tc.tile_pool(name="persist", bufs=1) as persist,
            tc.tile_pool(name="pt", bufs=3) as ptp,
            tc.tile_pool(name="misc", bufs=2) as misc,
        ):
            x_sb = persist.tile([128, CB, NI], mm_dt, tag="x")
            ctx_sb = persist.tile([128, CB, CTX_LEN], mm_dt, tag="ctx")
            wq_sb = persist.tile([128, CB, INNER], mm_dt, tag="wq")
            wkv_sb = persist.tile([128, CB, 2 * INNER], mm_dt, tag="wkv")
            wout_sb = persist.tile([128, CB, DIM], mm_dt, tag="wout")
            bias_sb = persist.tile([128, CB], f32, tag="bias")
            ones32 = persist.tile([128, 128], f32, tag="ones32")
            ones_r = persist.tile([1, 64], mm_dt, tag="ones_r")
            qT_sb = persist.tile([128, CB, NI], mm_dt, tag="qT")
            kT_sb = persist.tile([128, CB, CTX_LEN], mm_dt, tag="kT")
            v_sb = persist.tile([128, JB, H * 65], mm_dt, tag="v")
            outT_sb = persist.tile([128, CB, NI], mm_dt, tag="outT")

            # ---- input loads, ordered so Q projection can start earliest ----
            nc.sync.dma_start(out=bias_sb, in_=bout_d.rearrange("(m p) -> p m", p=128))
            xr = x_d.rearrange("(c p) i -> p c i", p=128)
            cr = ctx_d.rearrange("(c p) j -> p c j", p=128)
            wqr = wq_d.rearrange("(c p) o -> p c o", p=128)
            wkvr = wkv_d.rearrange("(c p) o -> p c o", p=128)
            woutr = wout_d.rearrange("(c p) o -> p c o", p=128)
            for c in range(CB):
                nc.sync.dma_start(out=wq_sb[:, c, :], in_=wqr[:, c, :])
            for c in range(CB):
                nc.sync.dma_start(out=x_sb[:, c, :], in_=xr[:, c, :])
            for c in range(CB):
                nc.sync.dma_start(out=wkv_sb[:, c, :], in_=wkvr[:, c, :])
            for c in range(CB):
                nc.sync.dma_start(out=ctx_sb[:, c, :], in_=cr[:, c, :])
            for c in range(CB):
                nc.sync.dma_start(out=wout_sb[:, c, :], in_=woutr[:, c, :])

            # ones: fp32 memset -> rounded copy into v's ones columns
            nc.vector.memset(ones32, 1.0)
            nc.vector.tensor_copy(out=ones_r, in_=ones32[0:1, 0:64])
            wu = misc.tile([1, 1], f32, tag="wu")
            nc.scalar.activation(out=wu[:], in_=ones32[0:1, 0:1], func=Exp)
            v_cols = v_sb.rearrange("p j (h x) -> p j h x", h=H)
            nc.vector.tensor_copy(out=v_cols[:, :, :, 64:65], in_=ones32[:, 0:JB * H])

            # ---- projections (own PSUM pool, released before attention) ----
            with tc.tile_pool(name="psP", bufs=3, space="PSUM") as psP:
                for m in range(CB):
                    for it in range(IT):
                        ps = psP.tile([128, 512], f32, tag="mm")
                        for c in range(CB):
                            nc.tensor.matmul(
                                out=ps[:],
                                lhsT=wq_sb[:, c, m * 128:(m + 1) * 128],
                                rhs=x_sb[:, c, it * 512:(it + 1) * 512],
                                start=(c == 0), stop=(c == CB - 1),
                            )
                        nc.vector.tensor_copy(
                            out=qT_sb[:, m, it * 512:(it + 1) * 512], in_=ps[:])

                for jb in range(JB):
                    ps = psP.tile([128, 512], f32, tag="mm")
                    for c in range(CB):
                        nc.tensor.matmul(
                            out=ps[:],
                            lhsT=ctx_sb[:, c, jb * 128:(jb + 1) * 128],
                            rhs=wkv_sb[:, c, INNER:2 * INNER],
                            start=(c == 0), stop=(c == CB - 1),
                        )
                    nc.vector.tensor_copy(
                        out=v_cols[:, jb, :, 0:64],
                        in_=ps[:].rearrange("p (h x) -> p h x", h=H),
                    )

            # ---- attention: per head, 1024-wide exp, dual av accumulators.
            # K-projection for block mk+1 is emitted between head pairs so the
            # PE slack inside the ACT-bound attention loop hides it. ----
            with (
                tc.tile_pool(name="psS", bufs=2, space="PSUM") as psS,
                tc.tile_pool(name="psAV", bufs=3, space="PSUM") as psAV,
                tc.tile_pool(name="psK", bufs=1, space="PSUM") as psK,
            ):
                def kproj(m):
                    for jt in range(JT):
                        ps = psK.tile([128, 512], f32, tag="kmm", name=f"k_{m}_{jt}")
                        for c in range(CB):
                            nc.tensor.matmul(
                                out=ps[:],
                                lhsT=wkv_sb[:, c, m * 128:(m + 1) * 128],
                                rhs=ctx_sb[:, c, jt * 512:(jt + 1) * 512],
                                start=(c == 0), stop=(c == CB - 1),
                            )
                        nc.vector.tensor_copy(
                            out=kT_sb[:, m, jt * 512:(jt + 1) * 512], in_=ps[:])

                kproj(0)
                for h in range(H):
                    mk, r0 = h // 2, (h % 2) * 64
                    av = [psAV.tile([128, 512], f32, tag="av", name=f"av_{h}_{i}")
                          for i in range(IT)]
                    s_tiles = []

                    def s_pair(jb):
                        s = psS.tile([128, 1024], f32, tag="s")
                        for it in range(IT):
                            nc.tensor.matmul(
                                out=s[:, it * 512:(it + 1) * 512],
                                lhsT=kT_sb[r0:r0 + 64, mk, jb * 128:(jb + 1) * 128],
                                rhs=qT_sb[r0:r0 + 64, mk, it * 512:(it + 1) * 512],
                                start=True, stop=True,
                            )
                        s_tiles.append(s)

                    s_pair(0)
                    for jb in range(JB):
                        pt = ptp.tile([128, 1024], mm_dt, tag="pt")
                        nc.scalar.activation(out=pt[:], in_=s_tiles[jb][:], func=Exp)
                        if jb + 1 < JB:
                            s_pair(jb + 1)
                        for it in range(IT):
                            nc.tensor.matmul(
                                out=av[it][0:65, :],
                                lhsT=v_sb[:, jb, h * 65:(h + 1) * 65],
                                rhs=pt[:, it * 512:(it + 1) * 512],
                                start=(jb == 0), stop=(jb == JB - 1),
                            )

                    for it in range(IT):
                        rec32 = misc.tile([1, 512], f32, tag="rec32")
                        rec_r = misc.tile([1, 512], mm_dt, tag="rec_r")
                        nc.vector.reciprocal(out=rec32[:], in_=av[it][64:65, :])
                        nc.vector.tensor_copy(out=rec_r[:], in_=rec32[:])
                        bc = psAV.tile([64, 512], f32, tag="av",
                                       name=f"bc_{h}_{it}")
                        nc.tensor.matmul(out=bc[:], lhsT=ones_r[0:1, :],
                                         rhs=rec_r[:], start=True, stop=True)
                        av_sb = misc.tile([64, 512], f32, tag="av_sb")
                        nc.vector.tensor_copy(out=av_sb[:], in_=av[it][0:64, :])
                        nc.vector.tensor_mul(
                            out=outT_sb[r0:r0 + 64, mk, it * 512:(it + 1) * 512],
                            in0=av_sb[:], in1=bc[:])
                    if h % 2 == 1 and h // 2 + 1 < CB:
                        kproj(h // 2 + 1)

            # ---- output projection + bias -> staged chunks -> DRAM ----
            with tc.tile_pool(name="psO", bufs=2, space="PSUM") as psO:
                for m in range(CB):
                    for it in range(IT):
                        ps = psO.tile([128, 512], f32, tag="mm")
                        for c in range(CB):
                            nc.tensor.matmul(
                                out=ps[:],
                                lhsT=wout_sb[:, c, m * 128:(m + 1) * 128],
                                rhs=outT_sb[:, c, it * 512:(it + 1) * 512],
                                start=(c == 0), stop=(c == CB - 1),
                            )
                        fin = ptp.tile([128, 512], f32, tag="fin")
                        nc.vector.tensor_scalar_add(
                            out=fin[:], in0=ps[:], scalar1=bias_sb[:, m:m + 1],
                        )
                        nc.sync.dma_start(
                            out=out_d[m * 128:(m + 1) * 128, it * 512:(it + 1) * 512],
                            in_=fin[:],
                        )

    nc.finalize()
    return nc


def _get_nc():
    if "nc" not in _CACHE:
        _CACHE["nc"] = _build()
    return _CACHE["nc"]


def kernel(x, context, W_q, W_kv, W_out, b_out, **_ignored):
    from concourse.bass_utils import run_bass_kernel_spmd

    x = np.asarray(x, dtype=np.float32)
    context = np.asarray(context, dtype=np.float32)
    wq = (np.asarray(W_q, dtype=np.float32) * np.float32(SCALE)).astype(np.float32)
    wkv = np.asarray(W_kv, dtype=np.float32)
    wout = np.asarray(W_out, dtype=np.float32)
    bout = np.asarray(b_out, dtype=np.float32)

    nc = _get_nc()
    in_maps = []
    for d in range(NCORES):
        b, half = d // 2, d % 2
        in_maps.append({
            "x": np.ascontiguousarray(x[b][:, half * NI:(half + 1) * NI]),
            "ctx": np.ascontiguousarray(context[b]),
            "wq": wq,
            "wkv": wkv,
            "wout": wout,
            "bout": bout,
        })

    res = run_bass_kernel_spmd(nc, in_maps, list(range(NCORES)))
    full = np.empty((B, DIM, N), dtype=np.float32)
    for d in range(NCORES):
        b, half = d // 2, d % 2
        full[b][:, half * NI:(half + 1) * NI] = res.results[d]["out"]
    return full



# revision 7
# speedup vs baseline: 1.2446x; 1.2446x over previous
"""Cross-attention kernel for Trainium2, sharded over 8 NeuronCores.

Problem (hardcoded shapes):
  x:       (4, 512, 2048)  queries, layout (b, dim, n)
  context: (4, 512, 2048)  keys/values source, layout (b, ctx_dim, m)
  W_q:     (512, 512), W_kv: (512, 1024), W_out: (512, 512), b_out: (512,)
  out = swapaxes(softmax((xs@Wq*scale) @ (cs@Wk)^T) @ (cs@Wv) @ Wout + b_out)

Sharding: 8 cores = 4 batches x 2 query-halves. Each core computes the
full 8-head attention for its (batch, 1024-query-slice) and produces the
exact output slice out[b][:, half] -- no cross-core reduction needed.

Per-core dataflow (bf16 inputs; kT/qT kept f32 and matmul'd as f32r,
which runs at the same PE rate as bf16 for >=256-wide outputs):
  qT[inner, i]  = Wq^T @ x_slice          (PE; Wq pre-scaled by 1/8;
                                           PSUM drained to SBUF by DMA)
  kT[inner, j]  = Wk^T @ ctx              (PE; DMA drain)
  v[j, h, dh+1] = ctx^T @ Wv              (PE; DVE copy to bf16, ones col)
  per head, streamed over 16 j-blocks:
      sT[j_blk, 0:1024] = kT_h^T @ qT_h      (PE, K=64)
      p = exp(sT) in bf16                    (split ACT exp / DVE
                                              Schraudolph bit-trick exp:
                                              bf16bits(e^s) ~ A*s + B)
      avT[i_blk, 0:65] += p_blk^T @ v_aug    (PE, bf16; i on partitions so
                                              the full 128-wide PE array is
                                              used; col 64 = sum_j p)
  norm: rec = 1/avT[:,64] (DVE), avn = avT[:,0:64]*rec (DVE per-partition
        scalar mul), then per head pair avn -> outT via XBAR DMA transpose.
  out[dim, i]  = Wout^T @ outT + b_out    (PE + DVE bias add)

DMA queueing: input loads trigger on SP; PSUM drains / transposes /
output stores trigger on the (otherwise idle) Pool engine's DGE.
"""

import os
import sys

sys.path.insert(0, "/opt/trn_rl_repo")

import numpy as np

B, DIM, N = 4, 512, 2048
CTX_DIM, CTX_LEN = 512, 2048
H, DH, INNER = 8, 64, 512
SCALE = DH ** -0.5

NCORES = 8
NI = 1024            # query rows per core
CB = DIM // 128      # 4 partition blocks of the feature/inner dims
IT = NI // 512       # 2 i-tiles (512 wide)
IC = NI // 128       # 8 i-chunks (128 wide)
JT = CTX_LEN // 512  # 4 j-tiles
JB = CTX_LEN // 128  # 16 j-blocks

# Schraudolph-style exp for bf16: bits_bf16(exp(s)) ~= round(A*s + B).
EXP_A = 184.66496523378733   # 128 / ln(2)
EXP_B = 16250.5              # 127*128 - 5.5 (minimax shift)

_CACHE = {}


def _exp_engine(h, jb):
    """Static schedule splitting the 128 exp blocks across ACT/DVE."""
    if h == 7:
        return "A" if jb % 3 == 0 else "D"
    return "D" if jb % 8 in (2, 5, 7) else "A"


def _build():
    import concourse.mybir as mybir
    from concourse import bacc
    from concourse.tile import TileContext

    f32 = mybir.dt.float32
    f32r = mybir.dt.float32r
    bf16 = mybir.dt.bfloat16
    i16 = mybir.dt.int16
    Exp = mybir.ActivationFunctionType.Exp
    Mult = mybir.AluOpType.mult
    Add = mybir.AluOpType.add

    nc = bacc.Bacc("TRN2", target_bir_lowering=False, debug=False)

    x_d = nc.dram_tensor("x", [DIM, NI], bf16, kind="ExternalInput").ap()
    ctx_d = nc.dram_tensor("ctx", [CTX_DIM, CTX_LEN], bf16, kind="ExternalInput").ap()
    wq_d = nc.dram_tensor("wq", [DIM, INNER], bf16, kind="ExternalInput").ap()
    wkv_d = nc.dram_tensor("wkv", [CTX_DIM, 2 * INNER], bf16, kind="ExternalInput").ap()
    wout_d = nc.dram_tensor("wout", [INNER, DIM], bf16, kind="ExternalInput").ap()
    bout_d = nc.dram_tensor("bout", [DIM], f32, kind="ExternalInput").ap()
    out_d = nc.dram_tensor("out", [DIM, NI], f32, kind="ExternalOutput").ap()

    with TileContext(nc) as tc:
        with (
            tc.tile_pool(name="persist", bufs=1) as persist,
            tc.tile_pool(name="ptall", bufs=2) as ptall,
            tc.tile_pool(name="misc", bufs=2) as misc,
            tc.tile_pool(name="avn", bufs=2) as avnp,
            tc.tile_pool(name="psS", bufs=2, space="PSUM") as psS,
            tc.tile_pool(name="psAV", bufs=2, space="PSUM") as psAV,
            tc.tile_pool(name="psM", bufs=2, space="PSUM") as psM,
        ):
            x_sb = persist.tile([128, CB, NI], bf16, tag="x")
            ctx_sb = persist.tile([128, CB, CTX_LEN], bf16, tag="ctx")
            wq_sb = persist.tile([128, CB, INNER], bf16, tag="wq")
            wkv_sb = persist.tile([128, CB, 2 * INNER], bf16, tag="wkv")
            wout_sb = persist.tile([128, CB, DIM], bf16, tag="wout")
            bias_sb = persist.tile([128, CB], f32, tag="bias")
            ones32 = persist.tile([128, 128], f32, tag="ones32")
            qT_sb = persist.tile([128, CB, NI], bf16, tag="qT")
            kT_sb = persist.tile([128, CB, CTX_LEN], bf16, tag="kT")
            v_sb = persist.tile([128, JB, H * 65], bf16, tag="v")
            outT_sb = persist.tile([128, CB, NI], bf16, tag="outT")

            # ---- input loads (SP queue), ordered so Q proj starts early ----
            nc.sync.dma_start(out=bias_sb, in_=bout_d.rearrange("(m p) -> p m", p=128))
            xr = x_d.rearrange("(c p) i -> p c i", p=128)
            cr = ctx_d.rearrange("(c p) j -> p c j", p=128)
            wqr = wq_d.rearrange("(c p) o -> p c o", p=128)
            wkvr = wkv_d.rearrange("(c p) o -> p c o", p=128)
            woutr = wout_d.rearrange("(c p) o -> p c o", p=128)
            for c in range(CB):
                nc.sync.dma_start(out=wq_sb[:, c, :], in_=wqr[:, c, :])
            for c in range(CB):
                nc.sync.dma_start(out=x_sb[:, c, :], in_=xr[:, c, :])
            for c in range(CB):
                nc.sync.dma_start(out=wkv_sb[:, c, :], in_=wkvr[:, c, :])
            for c in range(CB):
                nc.sync.dma_start(out=ctx_sb[:, c, :], in_=cr[:, c, :])
            for c in range(CB):
                nc.sync.dma_start(out=wout_sb[:, c, :], in_=woutr[:, c, :])

            # ones column of v (bf16) + ACT exp table warmup
            nc.vector.memset(ones32, 1.0)
            v_cols = v_sb.rearrange("p j (h x) -> p j h x", h=H)
            nc.vector.tensor_copy(out=v_cols[:, :, :, 64:65], in_=ones32[:, 0:JB * H])
            wu = misc.tile([1, 1], f32, tag="wu")
            nc.scalar.activation(out=wu[:], in_=ones32[0:1, 0:1], func=Exp)

            def qproj(m):
                for it in range(IT):
                    ps = psM.tile([128, 512], f32, tag="mm", name=f"q_{m}_{it}")
                    for c in range(CB):
                        nc.tensor.matmul(
                            out=ps[:],
                            lhsT=wq_sb[:, c, m * 128:(m + 1) * 128],
                            rhs=x_sb[:, c, it * 512:(it + 1) * 512],
                            start=(c == 0), stop=(c == CB - 1),
                        )
                    nc.scalar.copy(
                        out=qT_sb[:, m, it * 512:(it + 1) * 512], in_=ps[:])

            def kproj(m):
                for jt in range(JT):
                    ps = psM.tile([128, 512], f32, tag="mm", name=f"k_{m}_{jt}")
                    for c in range(CB):
                        nc.tensor.matmul(
                            out=ps[:],
                            lhsT=wkv_sb[:, c, m * 128:(m + 1) * 128],
                            rhs=ctx_sb[:, c, jt * 512:(jt + 1) * 512],
                            start=(c == 0), stop=(c == CB - 1),
                        )
                    nc.scalar.copy(
                        out=kT_sb[:, m, jt * 512:(jt + 1) * 512], in_=ps[:])

            def vproj(jb):
                ps = psM.tile([128, 512], f32, tag="mm", name=f"v_{jb}")
                for c in range(CB):
                    nc.tensor.matmul(
                        out=ps[:],
                        lhsT=ctx_sb[:, c, jb * 128:(jb + 1) * 128],
                        rhs=wkv_sb[:, c, INNER:2 * INNER],
                        start=(c == 0), stop=(c == CB - 1),
                    )
                nc.vector.tensor_copy(
                    out=v_cols[:, jb, :, 0:64],
                    in_=ps[:].rearrange("p (h x) -> p h x", h=H),
                )

            qproj(0)
            kproj(0)

            def phase1(h):
                """S + exp for all 16 j-blocks of head h into pt_all."""
                mk, r0 = h // 2, (h % 2) * 64
                pa = ptall.tile([128, JB, NI], bf16, tag="ptall",
                                name=f"ptall_{h}")
                for jb in range(JB):
                    s = psS.tile([128, 1024], f32, tag="s", name=f"s_{h}_{jb}")
                    for it in range(IT):
                        nc.tensor.matmul(
                            out=s[:, it * 512:(it + 1) * 512],
                            lhsT=kT_sb[r0:r0 + 64, mk, jb * 128:(jb + 1) * 128],
                            rhs=qT_sb[r0:r0 + 64, mk, it * 512:(it + 1) * 512],
                            start=True, stop=True,
                        )
                    if h == 0:
                        vproj(jb)
                    if _exp_engine(h, jb) == "A":
                        nc.scalar.activation(out=pa[:, jb, :], in_=s[:], func=Exp)
                    else:
                        nc.vector.tensor_scalar(
                            out=pa[:, jb, :].bitcast(i16), in0=s[:],
                            scalar1=EXP_A, scalar2=EXP_B, op0=Mult, op1=Add,
                        )
                return pa

            def phase2(h, pa, avn_pair):
                """AV accumulation + normalization, one i-chunk at a time.

                PSUM allows one live accumulation group per 2KB bank, so the
                8 groups ping-pong over the 2 psAV slots; the pool's WAR
                tracking orders group ic+2 after the normalize of group ic.
                """
                mk, r0 = h // 2, (h % 2) * 64
                for ic in range(IC):
                    av = psAV.tile([128, 512], f32, tag="av",
                                   name=f"av_{h}_{ic}")
                    for jb in range(JB):
                        nc.tensor.matmul(
                            out=av[:, 0:65],
                            lhsT=pa[:, jb, ic * 128:(ic + 1) * 128],
                            rhs=v_sb[:, jb, h * 65:(h + 1) * 65],
                            start=(jb == 0), stop=(jb == JB - 1),
                        )
                    rec = misc.tile([128, 1], f32, tag="rec",
                                    name=f"rec_{h}_{ic}")
                    nc.vector.reciprocal(out=rec[:], in_=av[:, 64:65])
                    nc.vector.tensor_scalar_mul(
                        out=avn_pair[:, ic, r0:r0 + 64],
                        in0=av[:, 0:64],
                        scalar1=rec[:],
                    )

            avn_pair = None
            pa = phase1(0)
            for h in range(H):
                mk = h // 2
                if h % 2 == 0:
                    avn_pair = avnp.tile([128, IC, 128], bf16, tag="avn",
                                         name=f"avn_{mk}")
                pa_next = phase1(h + 1) if h + 1 < H else None
                phase2(h, pa, avn_pair)
                pa = pa_next
                if h % 2 == 0:
                    if mk + 1 < CB:
                        qproj(mk + 1)
                        kproj(mk + 1)
                else:
                    for ic in range(IC):
                        nc.sync.dma_start_transpose(
                            out=outT_sb[:, mk, ic * 128:(ic + 1) * 128],
                            in_=avn_pair[:, ic, :])

            # ---- output projection + bias -> staged chunks -> DRAM ----
            for m in range(CB):
                for it in range(IT):
                    ps = psM.tile([128, 512], f32, tag="mm", name=f"o_{m}_{it}")
                    for c in range(CB):
                        nc.tensor.matmul(
                            out=ps[:],
                            lhsT=wout_sb[:, c, m * 128:(m + 1) * 128],
                            rhs=outT_sb[:, c, it * 512:(it + 1) * 512],
                            start=(c == 0), stop=(c == CB - 1),
                        )
                    fin = misc.tile([128, 512], f32, tag="fin")
                    nc.vector.tensor_scalar_add(
                        out=fin[:], in0=ps[:], scalar1=bias_sb[:, m:m + 1],
                    )
                    nc.sync.dma_start(
                        out=out_d[m * 128:(m + 1) * 128, it * 512:(it + 1) * 512],
                        in_=fin[:],
                    )

    nc.finalize()
    return nc


def _get_nc():
    if "nc" not in _CACHE:
        _CACHE["nc"] = _build()
    return _CACHE["nc"]


def make_in_maps(x, context, W_q, W_kv, W_out, b_out):
    import ml_dtypes

    bf16 = ml_dtypes.bfloat16
    x = np.asarray(x, dtype=np.float32)
    context = np.asarray(context, dtype=np.float32)
    wq = (np.asarray(W_q, dtype=np.float32) * np.float32(SCALE)).astype(bf16)
    wkv = np.asarray(W_kv, dtype=np.float32).astype(bf16)
    wout = np.asarray(W_out, dtype=np.float32).astype(bf16)
    bout = np.asarray(b_out, dtype=np.float32)

    in_maps = []
    for d in range(NCORES):
        b, half = d // 2, d % 2
        in_maps.append({
            "x": np.ascontiguousarray(
                x[b][:, half * NI:(half + 1) * NI]).astype(bf16),
            "ctx": np.ascontiguousarray(context[b]).astype(bf16),
            "wq": wq,
            "wkv": wkv,
            "wout": wout,
            "bout": bout,
        })
    return in_maps


def kernel(x, context, W_q, W_kv, W_out, b_out, **_ignored):
    from concourse.bass_utils import run_bass_kernel_spmd

    nc = _get_nc()
    in_maps = make_in_maps(x, context, W_q, W_kv, W_out, b_out)
    res = run_bass_kernel_spmd(nc, in_maps, list(range(NCORES)))
    full = np.empty((B, DIM, N), dtype=np.float32)
    for d in range(NCORES):
        b, half = d // 2, d % 2
        full[b][:, half * NI:(half + 1) * NI] = res.results[d]["out"]
    return full
